# revision 15
# baseline (speedup 1.0000x reference)
"""Trainium2 Bass kernel for CorrelatedSphericalField sampling (v6).

Math (validated against the jax reference):
  coeffs[t] = PHI^t * d_t,   d_t = d_{t-1} + PHI^{-t} * sigma_n (.) xi_{t-1},  d_0 = coeff0
  xs[t,n,k,m] = sum_l d[t,n,l,m] * pct[m,l,k]          (per-m Legendre GEMM)
  out[t,n,k,j] = 4pi * PHI^t * irfft_j(xs), as half-spectrum GEMMs:
      A[.., j] = sum_m xs_re[.., m] C[m, j],  B[.., j] = sum_m xs_im[.., m] S[m, j]
      out[.., 0:362] = A + B ;  out[.., 362+jj] = (A - B)[.., 360-jj]
  PHI^t and 4pi are folded into per-core C/S constants; sigma'*xi and c0 are
  host-folded into one slots tensor (slot0 = c0, slot t = PHI^{-t} sigma xi_{t-1})
  so stage A is just 7 in-place prefix adds.

Distribution (8 cores, single launch):
  stages A+B sharded over m (46 zero-padded m's per core, all (t,n)); two
  m-groups (16/30) pipeline stage B with a chunked AllToAll of xs (shard
  dim = t); stage D (iFFT GEMM over all 368 m-rows) is sharded over t.
  A2A payload layout [dst_t, m_local, e, n, k] makes the post-A2A loads six
  large contiguous DMAs, with C/S rows host-permuted to the matching m order.

Data is bf16 end to end (fp32 PSUM accumulation, fp32 output).
"""
import numpy as np
import ml_dtypes

import concourse.bass as bass
import concourse.mybir as mybir
import concourse.tile as tile
from concourse.bass_utils import run_bass_kernel_spmd

# ---- problem constants (hardcoded; kernel must be self-contained) ----
T = 8
N = 16
L = 361          # number of degrees l (contraction dim of stage B)
L2 = 384         # L zero-padded to 3*128
KLAT = 361       # number of latitudes
M = 362          # number of orders m
NLON = 722
JH = 362         # half-spectrum output columns of stage D
NC = 8
MPAD = 368       # M padded to a multiple of NC
MC = MPAD // NC  # 46 m's per core
TN = T * N       # 128
E = 2

PHI = float(np.exp(-6.0 / 48.0))
FOUR_PI = float(4.0 * np.pi)

LCH = [(0, 128), (128, 256), (256, 384)]
KCH = [(0, 128), (128, 256), (256, 361)]
# m-groups per core: sized so the global recv rows align exactly with the
# three 128-row stage-D contraction chunks (8*16=128 | 8*30=240=128+112)
MGRP = [(0, 16), (16, 46)]
G = len(MGRP)
ZOFF = [0, MGRP[0][1] * E * T * N]
ZW = MC * E * T * N          # 11776 per l-row
PCTB = 4                     # pct m's per load batch
DCH = [(0, 128), (128, 256), (256, 368)]  # stage-D contraction row chunks

F32 = mybir.dt.float32
BF16 = mybir.dt.bfloat16
NPBF = ml_dtypes.bfloat16


def _split_multi_waits(nc, max_inline=1):
    """The walrus build in this env accepts only one inline sync-wait per
    instruction; hoist extras onto same-engine NoOps placed just before."""
    ctr = 0
    for f in nc.m.functions:
        for bb in f.blocks:
            new = []
            for inst in bb.instructions:
                si = inst.sync_info
                if si is not None and si.on_wait and len(si.on_wait) > max_inline:
                    waits = list(si.on_wait)
                    keep = waits[-max_inline:]
                    for w in waits[:-max_inline]:
                        ctr += 1
                        nop = mybir.InstNoOp(name=f"I-wsplit-{ctr}",
                                             engine=inst.engine)
                        nop.sync_info = mybir.SyncInfo(on_wait=[w], on_update=[])
                        new.append(nop)
                    inst.sync_info = mybir.SyncInfo(
                        on_wait=keep, on_update=list(si.on_update))
                new.append(inst)
            bb.instructions = new


def build_nc(split_waits=True):
    nc = bass.Bass(num_devices=NC)

    # host layouts:
    #   z2  [l, group-packed (ml, e, slot, n)]  slot0=c0, slots1-7 = scaled xi
    #   pct [p(=l%128), m_local, lc(=l//128), k]
    #   csC/csS [368 rows in (group, src, ml) order, JH]
    z2_p = nc.declare_dram_parameter("z2", [L2, ZW], BF16, isOutput=False)
    pct_p = nc.declare_dram_parameter("pct_t", [128, MC, 3, KLAT], BF16,
                                      isOutput=False)
    csC_p = nc.declare_dram_parameter("csC", [MPAD, JH], BF16, isOutput=False)
    csS_p = nc.declare_dram_parameter("csS", [MPAD, JH], BF16, isOutput=False)
    out_p = nc.declare_dram_parameter("out_t", [N, KLAT, NLON], F32,
                                      isOutput=True)

    with tile.TileContext(nc) as tc:
        with tc.tile_pool(name="dram", bufs=1, space="DRAM") as pdram:
            sends, recvs = [], []
            for g, (ga, gb) in enumerate(MGRP):
                mg = gb - ga
                sends.append(pdram.tile([NC, mg, E, N, KLAT], BF16,
                                        name=f"send{g}", tag=f"send{g}"))
                recvs.append(pdram.tile([NC, mg, E, N, KLAT], BF16,
                                        name=f"recv{g}", tag=f"recv{g}"))

            with (
                tc.tile_pool(name="per", bufs=1) as pa,
                tc.tile_pool(name="cs", bufs=1) as pcs,
                tc.tile_pool(name="xr", bufs=1) as pxr,
                tc.tile_pool(name="w", bufs=3) as pw,
                tc.tile_pool(name="xs", bufs=8) as pxs,
                tc.tile_pool(name="psB", bufs=4, space="PSUM") as pp,
            ):
                d_tiles = {}

                # xr tiles for stage D: [rows, N*K] per (e, chunk)
                xr = {}
                for e in range(E):
                    for ch, (ra, rb) in enumerate(DCH):
                        xr[(e, ch)] = pxr.tile([rb - ra, N * KLAT], BF16,
                                               name=f"xr{e}{ch}",
                                               tag=f"xr{e}{ch}")

                # ---- stage B per group + scatter + big send + A2A ----------
                for g, (ga, gb) in enumerate(MGRP):
                    mg = gb - ga
                    # z2 load (sync ring) + stage A prefix sums for this group
                    for lc, (la, lb) in enumerate(LCH):
                        dt_ = pa.tile([128, mg * E, T, N], BF16,
                                      name=f"d{lc}g{g}", tag=f"d{lc}g{g}")
                        nc.sync.dma_start(
                            dt_[:].rearrange("p me t n -> p (me t n)"),
                            z2_p[la:lb, ZOFF[g]:ZOFF[g] + mg * E * T * N])
                        d_tiles[(lc, g)] = dt_
                    for lc in range(3):
                        dt_ = d_tiles[(lc, g)]
                        for t in range(1, T):
                            nc.vector.tensor_tensor(
                                out=dt_[:, :, t, :],
                                in0=dt_[:, :, t - 1, :],
                                in1=dt_[:, :, t, :],
                                op=mybir.AluOpType.add)
                    nb = (mg + PCTB - 1) // PCTB
                    for b in range(nb):
                        m0 = ga + b * PCTB
                        m1 = min(m0 + PCTB, gb)
                        w = pw.tile([128, PCTB, 3, KLAT], BF16, tag="pct")
                        peng = nc.sync if b % 2 == 0 else nc.scalar
                        peng.dma_start(w[:, 0:m1 - m0], pct_p[:, m0:m1])
                        for mi in range(m1 - m0):
                            ml = m0 + mi - ga
                            for e in range(E):
                                ps = pp.tile([TN, KLAT], F32, tag="ps")
                                for lc in range(3):
                                    nc.tensor.matmul(
                                        ps[:],
                                        d_tiles[(lc, g)][:, ml * E + e],
                                        w[:, mi, lc],
                                        start=(lc == 0), stop=(lc == 2))
                                xs_sb = pxs.tile([TN, KLAT], BF16, tag="xsb")
                                if e == 0:
                                    nc.scalar.copy(xs_sb[:], ps[:])
                                else:
                                    nc.vector.tensor_copy(xs_sb[:], ps[:])
                                nc.scalar.dma_start(
                                    sends[g][:, ml, e], xs_sb[:])

                # stage-D constants (scalar ring, after the send stream)
                csC_t, csS_t = [], []
                for ch, (ra, rb) in enumerate(DCH):
                    ct = pcs.tile([rb - ra, JH], BF16, name=f"csC{ch}",
                                  tag=f"csC{ch}")
                    st = pcs.tile([rb - ra, JH], BF16, name=f"csS{ch}",
                                  tag=f"csS{ch}")
                    nc.scalar.dma_start(ct[:], csC_p[ra:rb])
                    nc.scalar.dma_start(st[:], csS_p[ra:rb])
                    csC_t.append(ct)
                    csS_t.append(st)

                # collectives emitted AFTER all stage-B work: their only
                # data deps are the send DMAs, so they still trigger as soon
                # as each group's sends land, but no stage-B instruction can
                # pick up a transitive wait on the collective's completion.
                for g in range(G):
                    nc.gpsimd.collective_compute(
                        "AllToAll", mybir.AluOpType.bypass,
                        replica_groups=[list(range(NC))],
                        ins=[sends[g].opt()], outs=[recvs[g].opt()])
                    with tc.tile_wait_until(0.20 + 0.03 * g):
                        if g == 0:
                            for e in range(E):
                                nc.sync.dma_start(
                                    xr[(e, 0)][:],
                                    recvs[0][:, :, e].rearrange(
                                        "s m n k -> (s m) (n k)"))
                        else:
                            for e in range(E):
                                r1 = recvs[1][:, :, e].rearrange(
                                    "s m n k -> (s m) (n k)")
                                nc.sync.dma_start(xr[(e, 1)][:], r1[0:128])
                                nc.sync.dma_start(xr[(e, 2)][:], r1[128:240])

            # ---------------- stage D: iFFT GEMM over m ---------------------
            with (
                tc.tile_pool(name="o", bufs=4) as po,
                tc.tile_pool(name="psD", bufs=4, space="PSUM") as pp2,
            ):
                for n in range(N):
                    for (ka, kb) in KCH:
                        kp = kb - ka
                        psA = pp2.tile([kp, JH], F32, tag="psA")
                        psB = pp2.tile([kp, JH], F32, tag="psB")
                        for ch in range(3):
                            nc.tensor.matmul(
                                psA[:],
                                xr[(0, ch)][:, n * KLAT + ka:n * KLAT + kb],
                                csC_t[ch][:],
                                start=(ch == 0), stop=(ch == 2))
                        for ch in range(3):
                            nc.tensor.matmul(
                                psB[:],
                                xr[(1, ch)][:, n * KLAT + ka:n * KLAT + kb],
                                csS_t[ch][:],
                                start=(ch == 0), stop=(ch == 2))
                        oo = po.tile([kp, NLON], F32, tag="oo")
                        a_sb = po.tile([kp, JH], F32, tag="a_sb")
                        nc.scalar.copy(a_sb[:], psA[:])
                        nc.vector.tensor_tensor(
                            out=oo[:, 0:JH], in0=a_sb[:], in1=psB[:],
                            op=mybir.AluOpType.add)
                        nc.vector.tensor_tensor(
                            out=oo[:, JH:NLON], in0=a_sb[:, JH - 2:0:-1],
                            in1=psB[:, JH - 2:0:-1],
                            op=mybir.AluOpType.subtract)
                        nc.scalar.dma_start(out_p[n, ka:kb], oo[:])

    if split_waits:
        _split_multi_waits(nc)
    return nc


def prep_inputs(x, sigma_n, coeff0, xi, pct):
    """Host-side shard/stage: slice + transpose per-core inputs, build constants."""
    sigma_n = np.asarray(sigma_n, np.float32)
    coeff0 = np.asarray(coeff0, np.float32)
    xi = np.asarray(xi, np.float32)
    pct = np.asarray(pct, np.float32)

    padm = MPAD - M
    padl = L2 - L
    sig_pad = np.pad(sigma_n, ((0, padl), (0, padm)))
    c0_pad = np.pad(coeff0, ((0, 0), (0, padl), (0, padm), (0, 0)))
    xi_pad = np.pad(xi, ((0, 0), (0, 0), (0, padl), (0, padm), (0, 0)))
    pct_pad = np.pad(pct, ((0, padm), (0, padl), (0, 0)))

    # slots[s, n, l, m, e]: slot0 = c0, slot s = PHI^{-s} sigma (.) xi[s-1]
    phi_inv = (PHI ** -(np.arange(1, T, dtype=np.float64))).astype(np.float32)
    slots = np.empty((T, N, L2, MPAD, E), np.float32)
    slots[0] = c0_pad
    slots[1:] = (sig_pad[None, None, :, :, None] * xi_pad[:T - 1]
                 * phi_inv[:, None, None, None, None])

    # half-spectrum irfft matrices (fp64 host build)
    j = np.arange(JH, dtype=np.float64)
    mm = np.arange(M, dtype=np.float64)
    ang = 2.0 * np.pi * np.outer(mm, j) / NLON
    Cm = 2.0 * np.cos(ang)
    Cm[0, :] = 1.0
    Cm[M - 1, :] = np.cos(np.pi * j)
    Sm = -2.0 * np.sin(ang)
    Sm[0, :] = 0.0
    Sm[M - 1, :] = 0.0
    Cp = np.pad(Cm, ((0, padm), (0, 0)))
    Sp = np.pad(Sm, ((0, padm), (0, 0)))
    # stage-D row order: for each group, (src core, m_local) major
    order = np.array([src * MC + ga + ml
                      for (ga, gb) in MGRP
                      for src in range(NC)
                      for ml in range(gb - ga)], dtype=np.int64)
    Cp = Cp[order]
    Sp = Sp[order]

    in_maps = []
    for c in range(NC):
        msl = slice(c * MC, (c + 1) * MC)
        # [s, n, l, m, e] -> [l, m, e, s, n]
        z_c = np.transpose(slots[:, :, :, msl, :], (2, 3, 4, 0, 1))
        blocks = [np.ascontiguousarray(z_c[:, ga:gb]).reshape(L2, -1)
                  for (ga, gb) in MGRP]
        z2_c = np.concatenate(blocks, axis=1).astype(NPBF)
        # pct [m, l, k] -> [l%128, m, l//128, k]
        pct_c = np.ascontiguousarray(
            np.transpose(
                pct_pad[msl].reshape(MC, 3, 128, KLAT), (2, 0, 1, 3))
        ).astype(NPBF)
        scale = FOUR_PI * PHI ** c
        in_maps.append({
            "z2": z2_c,
            "pct_t": pct_c,
            "csC": (scale * Cp).astype(NPBF),
            "csS": (scale * Sp).astype(NPBF),
        })
    return in_maps


_NC_CACHE = None


def kernel(x, sigma_n, coeff0, xi, pct):
    global _NC_CACHE
    in_maps = prep_inputs(x, sigma_n, coeff0, xi, pct)
    if _NC_CACHE is None:
        _NC_CACHE = build_nc()
    res = run_bass_kernel_spmd(_NC_CACHE, in_maps, list(range(NC)))
    out = np.stack([res.results[c]["out_t"] for c in range(NC)], axis=0)
    return out.reshape(T, 1, 1, N, KLAT, NLON)


# revision 17
# speedup vs baseline: 1.0665x; 1.0665x over previous
"""Trainium2 Bass kernel for CorrelatedSphericalField sampling (v6).

Math (validated against the jax reference):
  coeffs[t] = PHI^t * d_t,   d_t = d_{t-1} + PHI^{-t} * sigma_n (.) xi_{t-1},  d_0 = coeff0
  xs[t,n,k,m] = sum_l d[t,n,l,m] * pct[m,l,k]          (per-m Legendre GEMM)
  out[t,n,k,j] = 4pi * PHI^t * irfft_j(xs), as half-spectrum GEMMs:
      A[.., j] = sum_m xs_re[.., m] C[m, j],  B[.., j] = sum_m xs_im[.., m] S[m, j]
      out[.., 0:362] = A + B ;  out[.., 362+jj] = (A - B)[.., 360-jj]
  PHI^t and 4pi are folded into per-core C/S constants; sigma'*xi and c0 are
  host-folded into one slots tensor (slot0 = c0, slot t = PHI^{-t} sigma xi_{t-1})
  so stage A is just 7 in-place prefix adds.

Distribution (8 cores, single launch):
  stages A+B sharded over m (46 zero-padded m's per core, all (t,n)); two
  m-groups (16/30) pipeline stage B with a chunked AllToAll of xs (shard
  dim = t); stage D (iFFT GEMM over all 368 m-rows) is sharded over t.
  A2A payload layout [dst_t, m_local, e, n, k] makes the post-A2A loads six
  large contiguous DMAs, with C/S rows host-permuted to the matching m order.

Data is bf16 end to end (fp32 PSUM accumulation, fp32 output).
"""
import numpy as np
import ml_dtypes

import concourse.bass as bass
import concourse.mybir as mybir
import concourse.tile as tile
from concourse.bass_utils import run_bass_kernel_spmd

# ---- problem constants (hardcoded; kernel must be self-contained) ----
T = 8
N = 16
L = 361          # number of degrees l (contraction dim of stage B)
L2 = 384         # L zero-padded to 3*128
KLAT = 361       # number of latitudes
M = 362          # number of orders m
NLON = 722
JH = 362         # half-spectrum output columns of stage D
NC = 8
MPAD = 368       # M padded to a multiple of NC
MC = MPAD // NC  # 46 m's per core
TN = T * N       # 128
E = 2

PHI = float(np.exp(-6.0 / 48.0))
FOUR_PI = float(4.0 * np.pi)

LCH = [(0, 128), (128, 256), (256, 384)]
KCH = [(0, 128), (128, 256), (256, 361)]
# m-groups per core: sized so the global recv rows align exactly with the
# three 128-row stage-D contraction chunks (8*16=128 | 8*30=240=128+112)
MGRP = [(0, 16), (16, 46)]
G = len(MGRP)
ZOFF = [0, MGRP[0][1] * E * T * N]
ZW = MC * E * T * N          # 11776 per l-row
PCTB = 4                     # pct m's per load batch
DCH = [(0, 128), (128, 256), (256, 368)]  # stage-D contraction row chunks

F32 = mybir.dt.float32
BF16 = mybir.dt.bfloat16
NPBF = ml_dtypes.bfloat16


def _split_multi_waits(nc, max_inline=1):
    """The walrus build in this env accepts only one inline sync-wait per
    instruction; hoist extras onto same-engine NoOps placed just before."""
    ctr = 0
    for f in nc.m.functions:
        for bb in f.blocks:
            new = []
            for inst in bb.instructions:
                si = inst.sync_info
                if si is not None and si.on_wait and len(si.on_wait) > max_inline:
                    waits = list(si.on_wait)
                    keep = waits[-max_inline:]
                    for w in waits[:-max_inline]:
                        ctr += 1
                        nop = mybir.InstNoOp(name=f"I-wsplit-{ctr}",
                                             engine=inst.engine)
                        nop.sync_info = mybir.SyncInfo(on_wait=[w], on_update=[])
                        new.append(nop)
                    inst.sync_info = mybir.SyncInfo(
                        on_wait=keep, on_update=list(si.on_update))
                new.append(inst)
            bb.instructions = new


def build_nc(split_waits=True):
    nc = bass.Bass(num_devices=NC)

    # host layouts:
    #   z2  [l, group-packed (ml, e, slot, n)]  slot0=c0, slots1-7 = scaled xi
    #   pct [p(=l%128), m_local, lc(=l//128), k]
    #   csC/csS [368 rows in (group, src, ml) order, JH]
    z2_p = nc.declare_dram_parameter("z2", [L2, ZW], BF16, isOutput=False)
    pct_p = nc.declare_dram_parameter("pct_t", [128, MC, 3, KLAT], BF16,
                                      isOutput=False)
    csC_p = nc.declare_dram_parameter("csC", [MPAD, JH], BF16, isOutput=False)
    csS_p = nc.declare_dram_parameter("csS", [MPAD, JH], BF16, isOutput=False)
    out_p = nc.declare_dram_parameter("out_t", [N, KLAT, NLON], F32,
                                      isOutput=True)

    with tile.TileContext(nc) as tc:
        with tc.tile_pool(name="dram", bufs=1, space="DRAM") as pdram:
            sends, recvs = [], []
            for g, (ga, gb) in enumerate(MGRP):
                mg = gb - ga
                sends.append(pdram.tile([NC, mg, E, N, KLAT], BF16,
                                        name=f"send{g}", tag=f"send{g}"))
                recvs.append(pdram.tile([NC, mg, E, N, KLAT], BF16,
                                        name=f"recv{g}", tag=f"recv{g}"))

            with (
                tc.tile_pool(name="per", bufs=1) as pa,
                tc.tile_pool(name="cs", bufs=1) as pcs,
                tc.tile_pool(name="xr", bufs=1) as pxr,
                tc.tile_pool(name="w", bufs=3) as pw,
                tc.tile_pool(name="xs", bufs=8) as pxs,
                tc.tile_pool(name="psB", bufs=4, space="PSUM") as pp,
            ):
                d_tiles = {}

                # xr tiles for stage D: [rows, N*K] per (e, chunk)
                xr = {}
                for e in range(E):
                    for ch, (ra, rb) in enumerate(DCH):
                        xr[(e, ch)] = pxr.tile([rb - ra, N * KLAT], BF16,
                                               name=f"xr{e}{ch}",
                                               tag=f"xr{e}{ch}")

                # ---- stage B per group + scatter + big send + A2A ----------
                for g, (ga, gb) in enumerate(MGRP):
                    mg = gb - ga
                    # z2 load (sync ring) + stage A prefix sums for this group
                    for lc, (la, lb) in enumerate(LCH):
                        dt_ = pa.tile([128, mg * E, T, N], BF16,
                                      name=f"d{lc}g{g}", tag=f"d{lc}g{g}")
                        nc.sync.dma_start(
                            dt_[:].rearrange("p me t n -> p (me t n)"),
                            z2_p[la:lb, ZOFF[g]:ZOFF[g] + mg * E * T * N])
                        d_tiles[(lc, g)] = dt_
                    for lc in range(3):
                        dt_ = d_tiles[(lc, g)]
                        for t in range(1, T):
                            nc.vector.tensor_tensor(
                                out=dt_[:, :, t, :],
                                in0=dt_[:, :, t - 1, :],
                                in1=dt_[:, :, t, :],
                                op=mybir.AluOpType.add)
                    nb = (mg + PCTB - 1) // PCTB
                    for b in range(nb):
                        m0 = ga + b * PCTB
                        m1 = min(m0 + PCTB, gb)
                        w = pw.tile([128, PCTB, 3, KLAT], BF16, tag="pct")
                        peng = (nc.scalar if (g == 0 and b % 2 == 1)
                                else nc.sync)
                        peng.dma_start(w[:, 0:m1 - m0], pct_p[:, m0:m1])
                        for mi in range(m1 - m0):
                            ml = m0 + mi - ga
                            for e in range(E):
                                ps = pp.tile([TN, KLAT], F32, tag="ps")
                                for lc in range(3):
                                    nc.tensor.matmul(
                                        ps[:],
                                        d_tiles[(lc, g)][:, ml * E + e],
                                        w[:, mi, lc],
                                        start=(lc == 0), stop=(lc == 2))
                                xs_sb = pxs.tile([TN, KLAT], BF16, tag="xsb")
                                if e == 0 and g == 0:
                                    nc.scalar.copy(xs_sb[:], ps[:])
                                else:
                                    nc.vector.tensor_copy(xs_sb[:], ps[:])
                                seng = nc.scalar if g == 0 else nc.sync
                                seng.dma_start(
                                    sends[g][:, ml, e], xs_sb[:])

                # stage-D constants (scalar ring, after the send stream)
                csC_t, csS_t = [], []
                for ch, (ra, rb) in enumerate(DCH):
                    ct = pcs.tile([rb - ra, JH], BF16, name=f"csC{ch}",
                                  tag=f"csC{ch}")
                    st = pcs.tile([rb - ra, JH], BF16, name=f"csS{ch}",
                                  tag=f"csS{ch}")
                    nc.sync.dma_start(ct[:], csC_p[ra:rb])
                    nc.sync.dma_start(st[:], csS_p[ra:rb])
                    csC_t.append(ct)
                    csS_t.append(st)

                # collectives emitted AFTER all stage-B work: their only
                # data deps are the send DMAs, so they still trigger as soon
                # as each group's sends land, but no stage-B instruction can
                # pick up a transitive wait on the collective's completion.
                for g in range(G):
                    nc.gpsimd.collective_compute(
                        "AllToAll", mybir.AluOpType.bypass,
                        replica_groups=[list(range(NC))],
                        ins=[sends[g].opt()], outs=[recvs[g].opt()])
                    _prio = tc.cur_priority
                    tc.cur_priority = _prio + 50000
                    if True:
                        if g == 0:
                            for e in range(E):
                                nc.sync.dma_start(
                                    xr[(e, 0)][:],
                                    recvs[0][:, :, e].rearrange(
                                        "s m n k -> (s m) (n k)"))
                        else:
                            for e in range(E):
                                r1 = recvs[1][:, :, e].rearrange(
                                    "s m n k -> (s m) (n k)")
                                nc.sync.dma_start(xr[(e, 1)][:], r1[0:128])
                                nc.sync.dma_start(xr[(e, 2)][:], r1[128:240])

            # ---------------- stage D: iFFT GEMM over m ---------------------
            with (
                tc.tile_pool(name="o", bufs=4) as po,
                tc.tile_pool(name="psD", bufs=4, space="PSUM") as pp2,
            ):
                for n in range(N):
                    for (ka, kb) in KCH:
                        kp = kb - ka
                        psA = pp2.tile([kp, JH], F32, tag="psA")
                        psB = pp2.tile([kp, JH], F32, tag="psB")
                        for ch in range(3):
                            nc.tensor.matmul(
                                psA[:],
                                xr[(0, ch)][:, n * KLAT + ka:n * KLAT + kb],
                                csC_t[ch][:],
                                start=(ch == 0), stop=(ch == 2))
                        for ch in range(3):
                            nc.tensor.matmul(
                                psB[:],
                                xr[(1, ch)][:, n * KLAT + ka:n * KLAT + kb],
                                csS_t[ch][:],
                                start=(ch == 0), stop=(ch == 2))
                        oo = po.tile([kp, NLON], F32, tag="oo")
                        a_sb = po.tile([kp, JH], F32, tag="a_sb")
                        nc.scalar.copy(a_sb[:], psA[:])
                        nc.vector.tensor_tensor(
                            out=oo[:, 0:JH], in0=a_sb[:], in1=psB[:],
                            op=mybir.AluOpType.add)
                        nc.vector.tensor_tensor(
                            out=oo[:, JH:NLON], in0=a_sb[:, JH - 2:0:-1],
                            in1=psB[:, JH - 2:0:-1],
                            op=mybir.AluOpType.subtract)
                        nc.scalar.dma_start(out_p[n, ka:kb], oo[:])

    if split_waits:
        _split_multi_waits(nc)
    return nc


def prep_inputs(x, sigma_n, coeff0, xi, pct):
    """Host-side shard/stage: slice + transpose per-core inputs, build constants."""
    sigma_n = np.asarray(sigma_n, np.float32)
    coeff0 = np.asarray(coeff0, np.float32)
    xi = np.asarray(xi, np.float32)
    pct = np.asarray(pct, np.float32)

    padm = MPAD - M
    padl = L2 - L
    sig_pad = np.pad(sigma_n, ((0, padl), (0, padm)))
    c0_pad = np.pad(coeff0, ((0, 0), (0, padl), (0, padm), (0, 0)))
    xi_pad = np.pad(xi, ((0, 0), (0, 0), (0, padl), (0, padm), (0, 0)))
    pct_pad = np.pad(pct, ((0, padm), (0, padl), (0, 0)))

    # slots[s, n, l, m, e]: slot0 = c0, slot s = PHI^{-s} sigma (.) xi[s-1]
    phi_inv = (PHI ** -(np.arange(1, T, dtype=np.float64))).astype(np.float32)
    slots = np.empty((T, N, L2, MPAD, E), np.float32)
    slots[0] = c0_pad
    slots[1:] = (sig_pad[None, None, :, :, None] * xi_pad[:T - 1]
                 * phi_inv[:, None, None, None, None])

    # half-spectrum irfft matrices (fp64 host build)
    j = np.arange(JH, dtype=np.float64)
    mm = np.arange(M, dtype=np.float64)
    ang = 2.0 * np.pi * np.outer(mm, j) / NLON
    Cm = 2.0 * np.cos(ang)
    Cm[0, :] = 1.0
    Cm[M - 1, :] = np.cos(np.pi * j)
    Sm = -2.0 * np.sin(ang)
    Sm[0, :] = 0.0
    Sm[M - 1, :] = 0.0
    Cp = np.pad(Cm, ((0, padm), (0, 0)))
    Sp = np.pad(Sm, ((0, padm), (0, 0)))
    # stage-D row order: for each group, (src core, m_local) major
    order = np.array([src * MC + ga + ml
                      for (ga, gb) in MGRP
                      for src in range(NC)
                      for ml in range(gb - ga)], dtype=np.int64)
    Cp = Cp[order]
    Sp = Sp[order]

    in_maps = []
    for c in range(NC):
        msl = slice(c * MC, (c + 1) * MC)
        # [s, n, l, m, e] -> [l, m, e, s, n]
        z_c = np.transpose(slots[:, :, :, msl, :], (2, 3, 4, 0, 1))
        blocks = [np.ascontiguousarray(z_c[:, ga:gb]).reshape(L2, -1)
                  for (ga, gb) in MGRP]
        z2_c = np.concatenate(blocks, axis=1).astype(NPBF)
        # pct [m, l, k] -> [l%128, m, l//128, k]
        pct_c = np.ascontiguousarray(
            np.transpose(
                pct_pad[msl].reshape(MC, 3, 128, KLAT), (2, 0, 1, 3))
        ).astype(NPBF)
        scale = FOUR_PI * PHI ** c
        in_maps.append({
            "z2": z2_c,
            "pct_t": pct_c,
            "csC": (scale * Cp).astype(NPBF),
            "csS": (scale * Sp).astype(NPBF),
        })
    return in_maps


_NC_CACHE = None


def kernel(x, sigma_n, coeff0, xi, pct):
    global _NC_CACHE
    in_maps = prep_inputs(x, sigma_n, coeff0, xi, pct)
    if _NC_CACHE is None:
        _NC_CACHE = build_nc()
    res = run_bass_kernel_spmd(_NC_CACHE, in_maps, list(range(NC)))
    out = np.stack([res.results[c]["out_t"] for c in range(NC)], axis=0)
    return out.reshape(T, 1, 1, N, KLAT, NLON)
